# revision 4
# baseline (speedup 1.0000x reference)
"""GumbelQuantizer forward on 8 Trainium2 NeuronCores.

Strategy (data-parallel over the bs*l token axis, per the sharding hint):
  - 32768 tokens are split into 8 shards of 4096 tokens; each core runs an
    identical Bass/Tile program on its shard. Weights + codebook replicated.
  - Per core:  hT = gelu(W1.T @ xT + b1)   (PE, f32r full-rate matmuls)
               logits = hT.T @ W2          (PE, token-major output)
               z = logits + (gumbels + b2) (DVE; b2 pre-folded into gumbels
                                            on host — mathematically identical)
               idx = argmax(z) per group   (DVE max/max_index)
               out = emb[idx]              (indirect-DMA gather per 128-token
                                            subtile and group)
  - The straight-through estimator's forward value is hard one-hot up to
    ~1.2e-7, so the softmax itself is skipped and the output is the gathered
    codebook row (exact fp32).
  - All DRAM operands are pre-swizzled on host into [128, ...] partition-major
    layouts so every DMA is 128 large contiguous descriptors (fast HWDGE
    descriptor generation, no ring backpressure).
  - W1 is sliced 4-ways / W2 2-ways so the first matmuls start as soon as the
    first ~512KB of weights land instead of after all weight DMA completes.
"""

import os
import sys

sys.path.insert(0, "/opt/trn_rl_repo")

import numpy as np

NCORES = 8
BS, L, DIM = 16, 2048, 512
NTOK = BS * L              # 32768 tokens total
TOK = NTOK // NCORES       # 4096 tokens per core
INNER = 1024
CODES = 320
G = 2
VD = 128                   # codebook row dim
CHUNK = 512                # tokens per pipeline chunk
NCHUNK = TOK // CHUNK      # 8
KT1 = DIM // 128           # 4  k-tiles for mm1
IT = INNER // 128          # 8  inner tiles
TT = CHUNK // 128          # 4  token sub-tiles per chunk
W1S = 4                    # W1 DMA slices (over inner blocks)
W1B = INNER // W1S // 128  # 2  128-wide inner blocks per slice
W2S = 2                    # W2 DMA slices (over k)
W2K = IT // W2S            # 4  k-tiles per W2 slice

_CACHE = {}


def _round_f32r(a: np.ndarray) -> np.ndarray:
    """Round fp32 values to the f32r grid (drop 12 mantissa bits, RNE)."""
    u = np.ascontiguousarray(a, np.float32).view(np.uint32).copy()
    low = u & 0xFFF
    keep = u & np.uint32(0xFFFFF000)
    round_up = (low > 0x800) | ((low == 0x800) & (((u >> 12) & 1) == 1))
    keep = keep + (round_up.astype(np.uint32) << 12)
    return keep.view(np.float32)


def _build_nc():
    import concourse.bass as bass
    import concourse.tile as tile
    from concourse import bacc, mybir

    f32 = mybir.dt.float32
    f32r = mybir.dt.float32r
    u32 = mybir.dt.uint32
    ADD = mybir.AluOpType.add
    GELU = mybir.ActivationFunctionType.Gelu

    nc = bacc.Bacc("TRN2")
    # Host-swizzled layouts: partition dim first, per-chunk slices contiguous.
    xA = nc.dram_tensor("xA", [128, NCHUNK, KT1, CHUNK], f32r,
                        kind="ExternalInput")
    gA = nc.dram_tensor("gA", [128, NCHUNK, TT, G, CODES], f32,
                        kind="ExternalInput")
    w1A = nc.dram_tensor("w1A", [128, W1S, KT1, W1B * 128], f32r,
                         kind="ExternalInput")
    w2A = nc.dram_tensor("w2A", [128, W2S, W2K, G * CODES], f32r,
                         kind="ExternalInput")
    b1A = nc.dram_tensor("b1A", [128, IT], f32, kind="ExternalInput")
    emb = nc.dram_tensor("emb", [CODES, VD], f32, kind="ExternalInput")
    outA = nc.dram_tensor("outA", [128, NCHUNK, TT, G * VD], f32,
                          kind="ExternalOutput")

    with tile.TileContext(nc) as tc:
        with (
            tc.tile_pool(name="consts", bufs=1) as consts,
            tc.tile_pool(name="xp", bufs=3) as xp,
            tc.tile_pool(name="hp", bufs=3) as hp,
            tc.tile_pool(name="gp", bufs=3) as gp,
            tc.tile_pool(name="zp", bufs=4) as zp,
            tc.tile_pool(name="mp", bufs=8) as mp,
            tc.tile_pool(name="op", bufs=2) as op,
            tc.tile_pool(name="ps1", bufs=2, space="PSUM") as ps1,
            tc.tile_pool(name="ps2", bufs=3, space="PSUM") as ps2,
        ):
            # --- weights first on the sync HWDGE queue (sliced: matmuls can
            # start after the first ~512KB lands); x/gum split across queues.
            w1s = []
            for s in range(W1S):
                w = consts.tile([128, KT1, W1B * 128], f32r, tag=f"w1_{s}")
                nc.sync.dma_start(w[:], w1A[:, s])
                w1s.append(w)
            w2s = []
            for s in range(W2S):
                w = consts.tile([128, W2K, G * CODES], f32r, tag=f"w2_{s}")
                nc.sync.dma_start(w[:], w2A[:, s])
                w2s.append(w)
            b1sb = consts.tile([128, IT], f32)
            nc.scalar.dma_start(b1sb[:], b1A[:])

            xsbs = {}
            gsbs = {}

            def load_inputs(ch):
                xsb = xp.tile([128, KT1, CHUNK], f32r)
                # odd chunks ride the sync queue (weights+out only there),
                # even chunks + gumbels ride the scalar queue
                if ch % 2 == 1:
                    nc.sync.dma_start(xsb[:], xA[:, ch])
                else:
                    nc.scalar.dma_start(xsb[:], xA[:, ch])
                xsbs[ch] = xsb
                gsb = gp.tile([128, TT, G, CODES], f32)
                nc.scalar.dma_start(gsb[:], gA[:, ch])
                gsbs[ch] = gsb

            def mm1_block(ch):
                # h[i] = gelu(W1[:, i].T @ x + b1[i])
                xsb = xsbs[ch]
                hs = []
                for i in range(IT):
                    s, j = divmod(i, W1B)
                    ph = ps1.tile([128, CHUNK], f32)
                    for k in range(KT1):
                        nc.tensor.matmul(
                            ph[:],
                            w1s[s][:, k, j * 128:(j + 1) * 128],
                            xsb[:, k, :],
                            start=(k == 0),
                            stop=(k == KT1 - 1),
                        )
                    h = hp.tile([128, CHUNK], f32r, tag=f"h{i}")
                    nc.scalar.activation(h[:], ph[:], GELU,
                                         bias=b1sb[:, i:i + 1])
                    hs.append(h)
                return hs

            def mm2_block(ch, hs):
                gsb = gsbs[ch]
                osb = op.tile([128, TT, G * VD], f32)
                for t in range(TT):
                    pz = ps2.tile([128, G, 512], f32)
                    for k in range(IT):
                        s, kl = divmod(k, W2K)
                        for g2 in range(G):
                            nc.tensor.matmul(
                                pz[:, g2, 0:CODES],
                                hs[k][:, t * 128:(t + 1) * 128],
                                w2s[s][:, kl, g2 * CODES:(g2 + 1) * CODES],
                                start=(k == 0),
                                stop=(k == IT - 1),
                            )
                    zsb = zp.tile([128, G, CODES], f32)
                    nc.vector.tensor_tensor(zsb[:], pz[:, :, 0:CODES],
                                            gsb[:, t], op=ADD)
                    for g2 in range(G):
                        m8 = mp.tile([128, 8], f32, tag="m8")
                        mi = mp.tile([128, 8], u32, tag="mi")
                        nc.vector.max(m8[:], zsb[:, g2, :])
                        nc.vector.max_index(mi[:], m8[:], zsb[:, g2, :])
                        nc.gpsimd.indirect_dma_start(
                            out=osb[:, t, g2 * VD:(g2 + 1) * VD],
                            out_offset=None,
                            in_=emb[:],
                            in_offset=bass.IndirectOffsetOnAxis(ap=mi[:, 0:1],
                                                                axis=0),
                        )
                nc.sync.dma_start(outA[:, ch], osb[:])

            # software pipeline: mm1 runs one chunk ahead of mm2 so PE never
            # waits on W2/gumbels and DMA hiccups don't re-throttle the clock
            load_inputs(0)
            hs_cur = mm1_block(0)
            for ch in range(NCHUNK):
                if ch + 1 < NCHUNK:
                    load_inputs(ch + 1)
                    hs_next = mm1_block(ch + 1)
                else:
                    hs_next = None
                mm2_block(ch, hs_cur)
                hs_cur = hs_next

    nc.compile()
    return nc


def kernel(**inputs) -> np.ndarray:
    from concourse.bass_utils import run_bass_kernel_spmd

    x = np.asarray(inputs["x"], np.float32)
    gumbels = np.asarray(inputs["gumbels"], np.float32)
    W1 = np.asarray(inputs["W1"], np.float32)
    b1 = np.asarray(inputs["b1"], np.float32)
    W2 = np.asarray(inputs["W2"], np.float32)
    b2 = np.asarray(inputs["b2"], np.float32)
    emb = np.asarray(inputs["emb"], np.float32)

    if "nc" not in _CACHE:
        _CACHE["nc"] = _build_nc()
    nc = _CACHE["nc"]

    xt = x.reshape(NTOK, DIM)
    # weight swizzles: [128, slice, k, cols] partition-major contiguous
    W1r = _round_f32r(W1)
    w1A = np.ascontiguousarray(
        W1r.reshape(KT1, 128, W1S, W1B * 128).transpose(1, 2, 0, 3))
    W2r = _round_f32r(W2)
    w2A = np.ascontiguousarray(
        W2r.reshape(W2S, W2K, 128, G * CODES).transpose(2, 0, 1, 3))
    b1A = np.ascontiguousarray(b1.reshape(IT, 128).T)
    # fold b2 into the gumbel noise: z = logits + b2 + gumbels
    gumb = gumbels.reshape(NTOK, G, CODES) + b2.reshape(G, CODES)

    in_maps = []
    for c in range(NCORES):
        xs = _round_f32r(xt[c * TOK:(c + 1) * TOK])
        xA = np.ascontiguousarray(
            xs.reshape(NCHUNK, CHUNK, KT1, 128).transpose(3, 0, 2, 1))
        gs = gumb[c * TOK:(c + 1) * TOK]
        gA = np.ascontiguousarray(
            gs.reshape(NCHUNK, TT, 128, G, CODES).transpose(2, 0, 1, 3, 4))
        in_maps.append({
            "xA": xA,
            "gA": gA,
            "w1A": w1A,
            "w2A": w2A,
            "b1A": b1A,
            "emb": emb,
        })

    trace = bool(int(os.environ.get("KERNEL_TRACE", "0")))
    res = run_bass_kernel_spmd(nc, in_maps, core_ids=list(range(NCORES)),
                               trace=trace)
    _CACHE["last_result"] = res
    outs = []
    for c in range(NCORES):
        o = res.results[c]["outA"]  # [128, NCHUNK, TT, 256]
        outs.append(o.transpose(1, 2, 0, 3).reshape(TOK, G * VD))
    return np.concatenate(outs, axis=0).reshape(BS, L, G * VD)


# revision 13
# speedup vs baseline: 1.0268x; 1.0268x over previous
"""GumbelQuantizer forward on 8 Trainium2 NeuronCores.

Strategy (data-parallel over the bs*l token axis, per the sharding hint):
  - 32768 tokens are split into 8 shards of 4096 tokens; each core runs an
    identical Bass/Tile program on its shard. Weights + codebook replicated.
  - Per core:  hT = gelu(W1.T @ xT + b1)   (PE, f32r full-rate matmuls)
               logits = hT.T @ W2          (PE, token-major output)
               z = logits + (gumbels + b2) (DVE; b2 pre-folded into gumbels
                                            on host — mathematically identical)
               idx = argmax(z) per group   (DVE max/max_index)
               out = emb[idx]              (indirect-DMA gather per 128-token
                                            subtile and group)
  - The straight-through estimator's forward value is hard one-hot up to
    ~1.2e-7, so the softmax itself is skipped and the output is the gathered
    codebook row (exact fp32).
  - All DRAM operands are pre-swizzled on host into [128, ...] partition-major
    layouts so every DMA is 128 large contiguous descriptors (fast HWDGE
    descriptor generation, no ring backpressure).
  - W1 is sliced 4-ways / W2 2-ways so the first matmuls start as soon as the
    first ~512KB of weights land instead of after all weight DMA completes.
"""

import os
import sys

sys.path.insert(0, "/opt/trn_rl_repo")

import numpy as np

NCORES = 8
BS, L, DIM = 16, 2048, 512
NTOK = BS * L              # 32768 tokens total
TOK = NTOK // NCORES       # 4096 tokens per core
INNER = 1024
CODES = 320
G = 2
VD = 128                   # codebook row dim
CHUNK = 512                # tokens per pipeline chunk
NCHUNK = TOK // CHUNK      # 8
KT1 = DIM // 128           # 4  k-tiles for mm1
IT = INNER // 128          # 8  inner tiles
TT = CHUNK // 128          # 4  token sub-tiles per chunk
W1S = 4                    # W1 DMA slices (over inner blocks)
W1B = INNER // W1S // 128  # 2  128-wide inner blocks per slice
W2S = 2                    # W2 DMA slices (over k)
W2K = IT // W2S            # 4  k-tiles per W2 slice

_CACHE = {}


def _round_f32r(a: np.ndarray) -> np.ndarray:
    """Round fp32 values to the f32r grid (drop 12 mantissa bits, RNE)."""
    u = np.ascontiguousarray(a, np.float32).view(np.uint32).copy()
    low = u & 0xFFF
    keep = u & np.uint32(0xFFFFF000)
    round_up = (low > 0x800) | ((low == 0x800) & (((u >> 12) & 1) == 1))
    keep = keep + (round_up.astype(np.uint32) << 12)
    return keep.view(np.float32)


def _build_nc():
    import concourse.bass as bass
    import concourse.tile as tile
    from concourse import bacc, mybir

    f32 = mybir.dt.float32
    f32r = mybir.dt.float32r
    u32 = mybir.dt.uint32
    ADD = mybir.AluOpType.add
    GELU = mybir.ActivationFunctionType.Gelu

    nc = bacc.Bacc("TRN2")
    # Host-swizzled layouts: partition dim first, per-chunk slices contiguous.
    xA = nc.dram_tensor("xA", [128, NCHUNK, KT1, CHUNK], f32r,
                        kind="ExternalInput")
    gA = nc.dram_tensor("gA", [128, NCHUNK, TT, G, CODES], f32,
                        kind="ExternalInput")
    w1A = nc.dram_tensor("w1A", [128, W1S, KT1, W1B * 128], f32r,
                         kind="ExternalInput")
    w2A = nc.dram_tensor("w2A", [128, W2S, W2K, G * CODES], f32r,
                         kind="ExternalInput")
    b1A = nc.dram_tensor("b1A", [128, IT], f32, kind="ExternalInput")
    emb = nc.dram_tensor("emb", [CODES, VD], f32, kind="ExternalInput")
    outA = nc.dram_tensor("outA", [128, NCHUNK, TT, G * VD], f32,
                          kind="ExternalOutput")

    with tile.TileContext(nc) as tc:
        with (
            tc.tile_pool(name="consts", bufs=1) as consts,
            tc.tile_pool(name="xp", bufs=3) as xp,
            tc.tile_pool(name="hp", bufs=3) as hp,
            tc.tile_pool(name="gp", bufs=3) as gp,
            tc.tile_pool(name="zp", bufs=4) as zp,
            tc.tile_pool(name="mp", bufs=8) as mp,
            tc.tile_pool(name="op", bufs=2) as op,
            tc.tile_pool(name="ps1", bufs=2, space="PSUM") as ps1,
            tc.tile_pool(name="ps2", bufs=3, space="PSUM") as ps2,
        ):
            # --- PE warm-up: ~12 dummy matmuls on a memset tile keep the PE
            # busy through the HAM activity window while the first input DMAs
            # land, so real matmuls start at 2.4GHz instead of 1.2GHz.
            dummy = consts.tile([128, CHUNK], f32, tag="dummy")
            nc.vector.memset(dummy[:], 0.0)
            dummy_r = dummy[:].bitcast(f32r)
            pd = ps1.tile([128, CHUNK], f32, tag="ph")
            for _ in range(12):
                nc.tensor.matmul(pd[:], dummy_r[:, 0:128], dummy_r[:],
                                 start=True, stop=True)

            # --- startup-critical DMA order: x0 first on the sync ring,
            # w1_0/w1_1 on the scalar ring, the rest behind them.
            xsbs = {}
            gsbs = {}
            x0sb = xp.tile([128, KT1, CHUNK], f32r, tag="x", name="x0sb")
            nc.sync.dma_start(x0sb[:], xA[:, 0])
            xsbs[0] = x0sb
            w1s = []
            for s in range(W1S):
                w = consts.tile([128, KT1, W1B * 128], f32r, tag=f"w1_{s}")
                (nc.scalar if s < 2 else nc.sync).dma_start(w[:], w1A[:, s])
                w1s.append(w)
            w2s = []
            for s in range(W2S):
                w = consts.tile([128, W2K, G * CODES], f32r, tag=f"w2_{s}")
                nc.sync.dma_start(w[:], w2A[:, s])
                w2s.append(w)
            b1sb = consts.tile([128, IT], f32)
            nc.scalar.dma_start(b1sb[:], b1A[:])

            def load_x(ch):
                if ch in xsbs or ch >= NCHUNK:
                    return
                xsb = xp.tile([128, KT1, CHUNK], f32r, tag="x")
                # alternate rings: even chunks sync, odd chunks scalar
                if ch % 2 == 0:
                    nc.sync.dma_start(xsb[:], xA[:, ch])
                else:
                    nc.scalar.dma_start(xsb[:], xA[:, ch])
                xsbs[ch] = xsb

            def load_g(ch):
                gsb = gp.tile([128, TT, G, CODES], f32)
                nc.scalar.dma_start(gsb[:], gA[:, ch])
                gsbs[ch] = gsb

            def mm1_block(ch):
                # h[i] = gelu(W1[:, i].T @ x + b1[i])
                xsb = xsbs[ch]
                hs = []
                for i in range(IT):
                    s, j = divmod(i, W1B)
                    ph = ps1.tile([128, CHUNK], f32, tag="ph")
                    for k in range(KT1):
                        nc.tensor.matmul(
                            ph[:],
                            w1s[s][:, k, j * 128:(j + 1) * 128],
                            xsb[:, k, :],
                            start=(k == 0),
                            stop=(k == KT1 - 1),
                        )
                    h = hp.tile([128, CHUNK], f32r, tag=f"h{i}")
                    nc.scalar.activation(h[:], ph[:], GELU,
                                         bias=b1sb[:, i:i + 1])
                    hs.append(h)
                return hs

            def mm2_block(ch, hs):
                gsb = gsbs[ch]
                osb = op.tile([128, TT, G, VD], f32)
                for t in range(TT):
                    pz = ps2.tile([128, G, 512], f32)
                    for k in range(IT):
                        s, kl = divmod(k, W2K)
                        for g2 in range(G):
                            nc.tensor.matmul(
                                pz[:, g2, 0:CODES],
                                hs[k][:, t * 128:(t + 1) * 128],
                                w2s[s][:, kl, g2 * CODES:(g2 + 1) * CODES],
                                start=(k == 0),
                                stop=(k == IT - 1),
                            )
                    zsb = zp.tile([128, G, CODES], f32)
                    nc.vector.tensor_tensor(zsb[:], pz[:, :, 0:CODES],
                                            gsb[:, t], op=ADD)
                    for g2 in range(G):
                        m8 = mp.tile([128, 8], f32, tag="m8")
                        mi = mp.tile([128, 8], u32, tag="mi")
                        nc.vector.max(m8[:], zsb[:, g2, :])
                        nc.vector.max_index(mi[:], m8[:], zsb[:, g2, :])
                        nc.gpsimd.indirect_dma_start(
                            out=osb[:, t, g2, :],
                            out_offset=None,
                            in_=emb[:],
                            in_offset=bass.IndirectOffsetOnAxis(ap=mi[:, 0:1],
                                                                axis=0),
                        )
                if ch == NCHUNK - 1:
                    # smaller tail: flush per token-subtile for the last chunk
                    for t in range(TT):
                        nc.sync.dma_start(outA[:, ch, t], osb[:, t])
                else:
                    nc.sync.dma_start(outA[:, ch], osb[:])

            # software pipeline: mm1 runs one chunk ahead of mm2 so PE never
            # waits on W2/gumbels and DMA hiccups don't re-throttle the clock
            load_x(1)
            load_g(0)
            hs_cur = mm1_block(0)
            for ch in range(NCHUNK):
                if ch + 1 < NCHUNK:
                    load_x(ch + 2)
                    load_g(ch + 1)
                    hs_next = mm1_block(ch + 1)
                else:
                    hs_next = None
                mm2_block(ch, hs_cur)
                hs_cur = hs_next

    nc.compile()
    return nc


def kernel(**inputs) -> np.ndarray:
    from concourse.bass_utils import run_bass_kernel_spmd

    x = np.asarray(inputs["x"], np.float32)
    gumbels = np.asarray(inputs["gumbels"], np.float32)
    W1 = np.asarray(inputs["W1"], np.float32)
    b1 = np.asarray(inputs["b1"], np.float32)
    W2 = np.asarray(inputs["W2"], np.float32)
    b2 = np.asarray(inputs["b2"], np.float32)
    emb = np.asarray(inputs["emb"], np.float32)

    if "nc" not in _CACHE:
        _CACHE["nc"] = _build_nc()
    nc = _CACHE["nc"]

    xt = x.reshape(NTOK, DIM)
    # weight swizzles: [128, slice, k, cols] partition-major contiguous
    W1r = _round_f32r(W1)
    w1A = np.ascontiguousarray(
        W1r.reshape(KT1, 128, W1S, W1B * 128).transpose(1, 2, 0, 3))
    W2r = _round_f32r(W2)
    w2A = np.ascontiguousarray(
        W2r.reshape(W2S, W2K, 128, G * CODES).transpose(2, 0, 1, 3))
    b1A = np.ascontiguousarray(b1.reshape(IT, 128).T)
    # fold b2 into the gumbel noise: z = logits + b2 + gumbels
    gumb = gumbels.reshape(NTOK, G, CODES) + b2.reshape(G, CODES)

    in_maps = []
    for c in range(NCORES):
        xs = _round_f32r(xt[c * TOK:(c + 1) * TOK])
        xA = np.ascontiguousarray(
            xs.reshape(NCHUNK, CHUNK, KT1, 128).transpose(3, 0, 2, 1))
        gs = gumb[c * TOK:(c + 1) * TOK]
        gA = np.ascontiguousarray(
            gs.reshape(NCHUNK, TT, 128, G, CODES).transpose(2, 0, 1, 3, 4))
        in_maps.append({
            "xA": xA,
            "gA": gA,
            "w1A": w1A,
            "w2A": w2A,
            "b1A": b1A,
            "emb": emb,
        })

    trace = bool(int(os.environ.get("KERNEL_TRACE", "0")))
    res = run_bass_kernel_spmd(nc, in_maps, core_ids=list(range(NCORES)),
                               trace=trace)
    _CACHE["last_result"] = res
    outs = []
    for c in range(NCORES):
        o = res.results[c]["outA"]  # [128, NCHUNK, TT, 256]
        outs.append(o.transpose(1, 2, 0, 3).reshape(TOK, G * VD))
    return np.concatenate(outs, axis=0).reshape(BS, L, G * VD)
